# revision 2
# baseline (speedup 1.0000x reference)
"""Trainium2 Bass kernel for the MoE routing problem.

Strategy (expert-parallel, host dispatch/combine):
  - Router runs on host in float64 (top-2 selection, softmax weights,
    aux load-balancing loss) — negligible FLOPs.
  - Core e computes its expert's swiglu over the tokens routed to it
    (capacity-padded so all 8 cores run one SPMD program), plus a 1/8
    token-slice of the shared expert.
  - Activations/weights are cast to bf16 on host; matmuls accumulate in
    fp32 PSUM; outputs return fp32; combine (routing-weight scatter-add)
    happens on host in fp32.

Device layout: everything transposed ([D, T]) so the matmul contraction
dim is always the SBUF partition dim and no on-device transposes are
needed (gate/up/down weight matrices are all naturally contraction-major).
"""

import os
import sys

import numpy as np

for _p in ("/opt/trn_rl_repo",):
    if _p not in sys.path and os.path.isdir(_p):
        sys.path.insert(0, _p)

import ml_dtypes

import concourse.bass as bass
import concourse.mybir as mybir
import concourse.tile as tile

BF16_NP = ml_dtypes.bfloat16

D = 1024
I = 2048
P = 128
KD = D // P  # 8
KI = I // P  # 16
NUM_EXPERTS = 8
TOP_K = 2
N_CORES = 8
BF16 = mybir.dt.bfloat16
F32 = mybir.dt.float32
AF = mybir.ActivationFunctionType

_CACHE = {}


def _split_multiwait_bir(bir_bytes):
    """This walrus build accepts only ONE sync-wait per instruction.
    Split any instruction with N>1 waits into N-1 preceding single-wait
    EventSemaphore instructions on the same engine (level-triggered
    sem-ge waits commute, so this is semantically identical)."""
    import json as _json

    j = _json.loads(bir_bytes)
    ctr = 0
    for fn in j["functions"]:
        for bb in fn["blocks"]:
            out = []
            for ins in bb["instructions"]:
                si = ins.get("sync_info")
                if si:
                    waits = si.get("on_wait") or []
                    if len(waits) > 1:
                        for w in waits[:-1]:
                            ctr += 1
                            out.append({
                                "debug": ins.get("debug", 0),
                                "engine": ins["engine"],
                                "ins": [],
                                "outs": [],
                                "name": f"{ins['name']}_sw{ctr}",
                                "opcode": "EventSemaphore",
                                "sync_info": {"on_update": [], "on_wait": [w]},
                            })
                        si["on_wait"] = [waits[-1]]
                out.append(ins)
            bb["instructions"] = out
    return _json.dumps(j).encode()


def _patch_compile():
    """Install the multi-wait splitter in front of walrus compilation."""
    import concourse.bass2jax as b2j

    if getattr(b2j, "_moe_multiwait_patched", False):
        return
    _orig = b2j.compile_bir_kernel

    def _cbk(bir_str, *a, **k):
        if isinstance(bir_str, str):
            bir_str = bir_str.encode()
        return _orig(_split_multiwait_bir(bir_str), *a, **k)

    b2j.compile_bir_kernel = _cbk
    b2j._moe_multiwait_patched = True


def _build_moe_nc(C_E: int, C_S: int, TB: int = 512, fused_silu: bool = True):
    assert C_E % P == 0 and C_S % P == 0
    nc = bass.Bass()

    xT_e = nc.declare_dram_parameter("xT_e", [D, C_E], BF16, isOutput=False)
    xT_s = nc.declare_dram_parameter("xT_s", [D, C_S], BF16, isOutput=False)
    params = {}
    for pfx in ("e", "s"):
        params[f"wg_{pfx}"] = nc.declare_dram_parameter(f"wg_{pfx}", [D, I], BF16, isOutput=False)
        params[f"wu_{pfx}"] = nc.declare_dram_parameter(f"wu_{pfx}", [D, I], BF16, isOutput=False)
        params[f"wd_{pfx}"] = nc.declare_dram_parameter(f"wd_{pfx}", [I, D], BF16, isOutput=False)
    yT_e = nc.declare_dram_parameter("yT_e", [D, C_E], F32, isOutput=True)
    yT_s = nc.declare_dram_parameter("yT_s", [D, C_S], F32, isOutput=True)

    with tile.TileContext(nc) as tc:
        with (
            tc.tile_pool(name="w", bufs=1) as wpool,
            tc.tile_pool(name="x", bufs=2) as xpool,
            tc.tile_pool(name="h", bufs=2) as hpool,
            tc.tile_pool(name="a", bufs=4) as apool,
            tc.tile_pool(name="zs", bufs=4) as zpool,
            tc.tile_pool(name="pgu", bufs=2, space="PSUM") as pgu,
            tc.tile_pool(name="pz", bufs=2, space="PSUM") as pzp,
        ):
            for pfx, xT, yT, C in (("e", xT_e, yT_e, C_E), ("s", xT_s, yT_s, C_S)):
                # Weights tiled [128 rows] x [quarter of the consumed dim] and
                # DMA'd in consumption order, so the PE is never weight-starved
                # at phase start and slots free quarter-by-quarter at the
                # phase switch. Weights ride the SP HWDGE queue alone.
                QD = KD // 4   # wd quarter: 2 dt-groups = 256 cols
                wg_sb = [[wpool.tile([P, 4 * P], BF16, tag=f"wg{k}_{q}", name=f"wg_sb_{pfx}_{k}_{q}")
                          for q in range(4)] for k in range(KD)]
                wu_sb = [[wpool.tile([P, 4 * P], BF16, tag=f"wu{k}_{q}", name=f"wu_sb_{pfx}_{k}_{q}")
                          for q in range(4)] for k in range(KD)]
                wd_sb = [[wpool.tile([P, QD * P], BF16, tag=f"wd{k}_{q}", name=f"wd_sb_{pfx}_{k}_{q}")
                          for q in range(4)] for k in range(KI)]
                for q in range(4):
                    for k in range(KD):
                        nc.sync.dma_start(out=wg_sb[k][q][:], in_=params[f"wg_{pfx}"][k * P:(k + 1) * P, q * 4 * P:(q + 1) * 4 * P])
                    for k in range(KD):
                        nc.sync.dma_start(out=wu_sb[k][q][:], in_=params[f"wu_{pfx}"][k * P:(k + 1) * P, q * 4 * P:(q + 1) * 4 * P])
                for q in range(4):
                    for k in range(KI):
                        nc.sync.dma_start(out=wd_sb[k][q][:], in_=params[f"wd_{pfx}"][k * P:(k + 1) * P, q * QD * P:(q + 1) * QD * P])

                for t0 in range(0, C, TB):
                    tb = min(TB, C - t0)
                    # x loads on the Activation HWDGE queue (don't queue
                    # behind the 12MB weight burst)
                    x_sb = [xpool.tile([P, TB], BF16, tag=f"x{k}", name=f"x_{pfx}_{t0}_{k}") for k in range(KD)]
                    for k in range(KD):
                        nc.scalar.dma_start(out=x_sb[k][:, :tb], in_=xT[k * P:(k + 1) * P, t0:t0 + tb])
                    h_sb = [hpool.tile([P, TB], BF16, tag=f"h{it}", name=f"h_{pfx}_{t0}_{it}") for it in range(KI)]
                    for it in range(KI):
                        pg = pgu.tile([P, TB], F32, tag="pg", name=f"pg_{pfx}_{t0}_{it}")
                        pu = pgu.tile([P, TB], F32, tag="pu", name=f"pu_{pfx}_{t0}_{it}")
                        for k in range(KD):
                            nc.tensor.matmul(
                                pg[:, :tb],
                                wg_sb[k][it // 4][:, (it % 4) * P:(it % 4 + 1) * P],
                                x_sb[k][:, :tb],
                                start=(k == 0), stop=(k == KD - 1),
                            )
                        for k in range(KD):
                            nc.tensor.matmul(
                                pu[:, :tb],
                                wu_sb[k][it // 4][:, (it % 4) * P:(it % 4 + 1) * P],
                                x_sb[k][:, :tb],
                                start=(k == 0), stop=(k == KD - 1),
                            )
                        h_slice = h_sb[it][:, :tb]
                        s_sb = apool.tile([P, TB], F32, tag="s", name=f"s_{pfx}_{t0}_{it}")
                        if fused_silu:
                            nc.scalar.activation(s_sb[:, :tb], pg[:, :tb], AF.Silu)
                            nc.vector.tensor_mul(h_slice, pu[:, :tb], s_sb[:, :tb])
                        else:
                            nc.scalar.activation(s_sb[:, :tb], pg[:, :tb], AF.Sigmoid)
                            t_sb = apool.tile([P, TB], F32, tag="t", name=f"t_{pfx}_{t0}_{it}")
                            nc.vector.tensor_mul(t_sb[:, :tb], pg[:, :tb], s_sb[:, :tb])
                            nc.vector.tensor_mul(h_slice, pu[:, :tb], t_sb[:, :tb])
                    for dt in range(KD):
                        pz = pzp.tile([P, TB], F32, tag="pz", name=f"pz_{pfx}_{t0}_{dt}")
                        for it in range(KI):
                            nc.tensor.matmul(
                                pz[:, :tb],
                                wd_sb[it][dt // 2][:, (dt % 2) * P:(dt % 2 + 1) * P],
                                h_sb[it][:, :tb],
                                start=(it == 0), stop=(it == KI - 1),
                            )
                        z_sb = zpool.tile([P, TB], F32, tag="z", name=f"z_{pfx}_{t0}_{dt}")
                        nc.vector.tensor_copy(out=z_sb[:, :tb], in_=pz[:, :tb])
                        # y stores via gpsimd SWDGE so they never delay x loads
                        # on the ACT HWDGE queue; final block on ACT (HWDGE
                        # drains faster at the tail barrier).
                        last_block = pfx == "s" and t0 + TB >= C
                        dma_eng = nc.scalar if last_block else nc.gpsimd
                        dma_eng.dma_start(out=yT[dt * P:(dt + 1) * P, t0:t0 + tb], in_=z_sb[:, :tb])
    return nc


def _ensure_ntff_hook():
    """The agent image's antenv lacks axon_hooks; synthesize it and
    register the ctypes NTFF profiling hook so trace=True works."""
    import types
    try:
        from antenv.axon_hooks import get_axon_ntff_profile_hook  # noqa: F401
        return
    except ImportError:
        pass
    mod = types.ModuleType("antenv.axon_hooks")
    mod._hook = None

    def set_axon_ntff_profile_hook(h):
        mod._hook = h

    def get_axon_ntff_profile_hook():
        return mod._hook

    mod.set_axon_ntff_profile_hook = set_axon_ntff_profile_hook
    mod.get_axon_ntff_profile_hook = get_axon_ntff_profile_hook
    sys.modules["antenv.axon_hooks"] = mod
    try:
        from trn_agent_boot.trn_boot import _ntff_profile_via_ctypes
        h = _ntff_profile_via_ctypes("/opt/axon/libaxon_pjrt.so")
        if h is not None:
            set_axon_ntff_profile_hook(h)
    except Exception as e:
        print(f"ntff hook setup failed: {e}", file=sys.stderr)


def _route(x, gate_w):
    """float64 routing: top-2 selection, softmax weights, aux loss."""
    T = x.shape[0]
    lg = x.astype(np.float64) @ gate_w.astype(np.float64).T  # [T, E]
    sel = np.argsort(-lg, axis=-1, kind="stable")[:, :TOP_K]  # [T, 2]
    top_vals = np.take_along_axis(lg, sel, axis=1)
    ex = np.exp(top_vals - top_vals.max(axis=1, keepdims=True))
    w = ex / ex.sum(axis=1, keepdims=True)  # [T, 2]

    counts = np.zeros(NUM_EXPERTS, dtype=np.int64)
    np.add.at(counts, sel.ravel(), 1)
    f = counts / (T * TOP_K)
    el = np.exp(lg - lg.max(axis=1, keepdims=True))
    Pm = (el / el.sum(axis=1, keepdims=True)).mean(axis=0)
    aux = np.float32(NUM_EXPERTS * np.sum(f * Pm))
    return sel, w.astype(np.float32), counts, aux


def kernel(hidden_states, gate_w, shared_gate, shared_up, shared_down,
           exp_gate, exp_up, exp_down):
    from concourse.bass_utils import run_bass_kernel_spmd

    B, S, Dm = hidden_states.shape
    x = np.ascontiguousarray(np.asarray(hidden_states, dtype=np.float32).reshape(-1, Dm))
    T = x.shape[0]

    sel, w, counts, aux = _route(x, np.asarray(gate_w, dtype=np.float32))

    tok_of = []
    wt_of = []
    for e in range(NUM_EXPERTS):
        mask = sel == e  # [T, 2]
        toks = np.nonzero(mask.any(axis=1))[0]
        we = np.where(mask[toks, 0], w[toks, 0], w[toks, 1])
        tok_of.append(toks)
        wt_of.append(we.astype(np.float32))

    C_E = max(128, int(-(-counts.max() // 128)) * 128)
    C_S = T // N_CORES

    key = (C_E, C_S)
    if key not in _CACHE:
        _patch_compile()
        _CACHE[key] = _build_moe_nc(C_E, C_S, TB=512)
    nc = _CACHE[key]

    xT_bf = np.ascontiguousarray(x.T).astype(BF16_NP)  # [D, T]

    in_maps = []
    for c in range(N_CORES):
        toks = tok_of[c]
        xT_e = np.zeros((D, C_E), dtype=BF16_NP)
        xT_e[:, :len(toks)] = xT_bf[:, toks]
        xT_s = np.ascontiguousarray(xT_bf[:, c * C_S:(c + 1) * C_S])
        in_maps.append({
            "xT_e": xT_e,
            "xT_s": xT_s,
            "wg_e": np.asarray(exp_gate[c], dtype=np.float32).astype(BF16_NP),
            "wu_e": np.asarray(exp_up[c], dtype=np.float32).astype(BF16_NP),
            "wd_e": np.asarray(exp_down[c], dtype=np.float32).astype(BF16_NP),
            "wg_s": np.asarray(shared_gate, dtype=np.float32).astype(BF16_NP),
            "wu_s": np.asarray(shared_up, dtype=np.float32).astype(BF16_NP),
            "wd_s": np.asarray(shared_down, dtype=np.float32).astype(BF16_NP),
        })

    trace = os.environ.get("MOE_TRACE") == "1"
    if trace:
        _ensure_ntff_hook()
    res = run_bass_kernel_spmd(nc, in_maps, core_ids=list(range(N_CORES)), trace=trace,
                               tmpdir=os.environ.get("MOE_TRACE_DIR") or None)
    kernel.last_exec_time_ns = res.exec_time_ns

    out = np.empty((T, Dm), dtype=np.float32)
    for c in range(N_CORES):
        out[c * C_S:(c + 1) * C_S] = res.results[c]["yT_s"].T
    for e in range(NUM_EXPERTS):
        toks = tok_of[e]
        y = res.results[e]["yT_e"][:, :len(toks)].T  # [n_e, D]
        out[toks] += wt_of[e][:, None] * y

    return out.reshape(B, S, Dm), np.float32(aux)


kernel.last_exec_time_ns = None


# revision 5
# speedup vs baseline: 1.1856x; 1.1856x over previous
"""Trainium2 Bass kernel for the MoE routing problem.

Strategy (expert-parallel, host dispatch/combine):
  - Router runs on host in float64 (top-2 selection, softmax weights,
    aux load-balancing loss) — negligible FLOPs.
  - Core e computes its expert's swiglu over the tokens routed to it
    (capacity-padded so all 8 cores run one SPMD program), plus a 1/8
    token-slice of the shared expert.
  - Activations/weights are cast to bf16 on host; matmuls accumulate in
    fp32 PSUM; outputs return fp32; combine (routing-weight scatter-add)
    happens on host in fp32.

Device layout: everything transposed ([D, T]) so the matmul contraction
dim is always the SBUF partition dim and no on-device transposes are
needed (gate/up/down weight matrices are all naturally contraction-major).
"""

import os
import sys

import numpy as np

for _p in ("/opt/trn_rl_repo",):
    if _p not in sys.path and os.path.isdir(_p):
        sys.path.insert(0, _p)

import ml_dtypes

import concourse.bass as bass
import concourse.mybir as mybir
import concourse.tile as tile

BF16_NP = ml_dtypes.bfloat16

D = 1024
I = 2048
P = 128
KD = D // P  # 8
KI = I // P  # 16
NUM_EXPERTS = 8
TOP_K = 2
N_CORES = 8
BF16 = mybir.dt.bfloat16
F32 = mybir.dt.float32
AF = mybir.ActivationFunctionType

_CACHE = {}


def _split_multiwait_bir(bir_bytes):
    """This walrus build accepts only ONE sync-wait per instruction.
    Split any instruction with N>1 waits into N-1 preceding single-wait
    EventSemaphore instructions on the same engine (level-triggered
    sem-ge waits commute, so this is semantically identical)."""
    import json as _json

    j = _json.loads(bir_bytes)
    ctr = 0
    for fn in j["functions"]:
        for bb in fn["blocks"]:
            out = []
            for ins in bb["instructions"]:
                si = ins.get("sync_info")
                if si:
                    waits = si.get("on_wait") or []
                    if len(waits) > 1:
                        for w in waits[:-1]:
                            ctr += 1
                            out.append({
                                "debug": ins.get("debug", 0),
                                "engine": ins["engine"],
                                "ins": [],
                                "outs": [],
                                "name": f"{ins['name']}_sw{ctr}",
                                "opcode": "EventSemaphore",
                                "sync_info": {"on_update": [], "on_wait": [w]},
                            })
                        si["on_wait"] = [waits[-1]]
                out.append(ins)
            bb["instructions"] = out
    return _json.dumps(j).encode()


def _patch_compile():
    """Install the multi-wait splitter in front of walrus compilation."""
    import concourse.bass2jax as b2j

    if getattr(b2j, "_moe_multiwait_patched", False):
        return
    _orig = b2j.compile_bir_kernel

    def _cbk(bir_str, *a, **k):
        if isinstance(bir_str, str):
            bir_str = bir_str.encode()
        return _orig(_split_multiwait_bir(bir_str), *a, **k)

    b2j.compile_bir_kernel = _cbk
    b2j._moe_multiwait_patched = True


def _build_moe_nc(C_E: int, C_S: int, TB: int = 512, fused_silu: bool = True):
    assert C_E % P == 0 and C_S % P == 0
    nc = bass.Bass()

    xT_e = nc.declare_dram_parameter("xT_e", [D, C_E], BF16, isOutput=False)
    xT_s = nc.declare_dram_parameter("xT_s", [D, C_S], BF16, isOutput=False)
    params = {}
    for pfx in ("e", "s"):
        params[f"wg_{pfx}"] = nc.declare_dram_parameter(f"wg_{pfx}", [D, I], BF16, isOutput=False)
        params[f"wu_{pfx}"] = nc.declare_dram_parameter(f"wu_{pfx}", [D, I], BF16, isOutput=False)
        params[f"wd_{pfx}"] = nc.declare_dram_parameter(f"wd_{pfx}", [I, D], BF16, isOutput=False)
    yT_e = nc.declare_dram_parameter("yT_e", [D, C_E], F32, isOutput=True)
    yT_s = nc.declare_dram_parameter("yT_s", [D, C_S], F32, isOutput=True)

    with tile.TileContext(nc) as tc:
        with (
            tc.tile_pool(name="w", bufs=1) as wpool,
            tc.tile_pool(name="x", bufs=2) as xpool,
            tc.tile_pool(name="h", bufs=2) as hpool,
            tc.tile_pool(name="a", bufs=4) as apool,
            tc.tile_pool(name="zs", bufs=4) as zpool,
            tc.tile_pool(name="pgu", bufs=3, space="PSUM") as pgu,
            tc.tile_pool(name="pz", bufs=2, space="PSUM") as pzp,
        ):
            for pfx, xT, yT, C in (("e", xT_e, yT_e, C_E), ("s", xT_s, yT_s, C_S)):
                # Weights tiled [128 rows] x [quarter of the consumed dim] and
                # DMA'd in consumption order, so the PE is never weight-starved
                # at phase start and slots free quarter-by-quarter at the
                # phase switch. Weights ride the SP HWDGE queue alone.
                QD = KD // 4   # wd quarter: 2 dt-groups = 256 cols
                wg_sb = [[wpool.tile([P, 4 * P], BF16, tag=f"wg{k}_{q}", name=f"wg_sb_{pfx}_{k}_{q}")
                          for q in range(4)] for k in range(KD)]
                wu_sb = [[wpool.tile([P, 4 * P], BF16, tag=f"wu{k}_{q}", name=f"wu_sb_{pfx}_{k}_{q}")
                          for q in range(4)] for k in range(KD)]
                wd_sb = [[wpool.tile([P, QD * P], BF16, tag=f"wd{k}_{q}", name=f"wd_sb_{pfx}_{k}_{q}")
                          for q in range(4)] for k in range(KI)]
                # First block's x loads go on the ACT queue BEFORE any
                # weight DMA lands there — the PE's very first matmuls
                # need x(b0) + wg q0 and nothing else.
                tb0 = min(TB, C)
                x0_sb = [xpool.tile([P, TB], BF16, tag=f"x{k}", name=f"x_{pfx}_0_{k}") for k in range(KD)]
                for k in range(KD):
                    nc.scalar.dma_start(out=x0_sb[k][:, :tb0], in_=xT[k * P:(k + 1) * P, 0:tb0])
                for q in range(4):
                    for k in range(KD):
                        nc.sync.dma_start(out=wg_sb[k][q][:], in_=params[f"wg_{pfx}"][k * P:(k + 1) * P, q * 4 * P:(q + 1) * 4 * P])
                    for k in range(KD):
                        nc.sync.dma_start(out=wu_sb[k][q][:], in_=params[f"wu_{pfx}"][k * P:(k + 1) * P, q * 4 * P:(q + 1) * 4 * P])
                for q in range(4):
                    for k in range(KI):
                        nc.sync.dma_start(out=wd_sb[k][q][:], in_=params[f"wd_{pfx}"][k * P:(k + 1) * P, q * QD * P:(q + 1) * QD * P])

                for t0 in range(0, C, TB):
                    tb = min(TB, C - t0)
                    # x loads on the Activation HWDGE queue (don't queue
                    # behind the 12MB weight burst)
                    if t0 == 0:
                        x_sb = x0_sb
                    else:
                        x_sb = [xpool.tile([P, TB], BF16, tag=f"x{k}", name=f"x_{pfx}_{t0}_{k}") for k in range(KD)]
                        for k in range(KD):
                            nc.scalar.dma_start(out=x_sb[k][:, :tb], in_=xT[k * P:(k + 1) * P, t0:t0 + tb])
                    h_sb = [hpool.tile([P, TB], BF16, tag=f"h{it}", name=f"h_{pfx}_{t0}_{it}") for it in range(KI)]
                    for it in range(KI):
                        pg = pgu.tile([P, TB], F32, tag="pg", name=f"pg_{pfx}_{t0}_{it}")
                        pu = pgu.tile([P, TB], F32, tag="pu", name=f"pu_{pfx}_{t0}_{it}")
                        for k in range(KD):
                            nc.tensor.matmul(
                                pg[:, :tb],
                                wg_sb[k][it // 4][:, (it % 4) * P:(it % 4 + 1) * P],
                                x_sb[k][:, :tb],
                                start=(k == 0), stop=(k == KD - 1),
                            )
                        for k in range(KD):
                            nc.tensor.matmul(
                                pu[:, :tb],
                                wu_sb[k][it // 4][:, (it % 4) * P:(it % 4 + 1) * P],
                                x_sb[k][:, :tb],
                                start=(k == 0), stop=(k == KD - 1),
                            )
                        h_slice = h_sb[it][:, :tb]
                        s_sb = apool.tile([P, TB], F32, tag="s", name=f"s_{pfx}_{t0}_{it}")
                        if fused_silu:
                            nc.scalar.activation(s_sb[:, :tb], pg[:, :tb], AF.Silu)
                            nc.vector.tensor_mul(h_slice, pu[:, :tb], s_sb[:, :tb])
                        else:
                            nc.scalar.activation(s_sb[:, :tb], pg[:, :tb], AF.Sigmoid)
                            t_sb = apool.tile([P, TB], F32, tag="t", name=f"t_{pfx}_{t0}_{it}")
                            nc.vector.tensor_mul(t_sb[:, :tb], pg[:, :tb], s_sb[:, :tb])
                            nc.vector.tensor_mul(h_slice, pu[:, :tb], t_sb[:, :tb])
                    for dt in range(KD):
                        pz = pzp.tile([P, TB], F32, tag="pz", name=f"pz_{pfx}_{t0}_{dt}")
                        for it in range(KI):
                            nc.tensor.matmul(
                                pz[:, :tb],
                                wd_sb[it][dt // 2][:, (dt % 2) * P:(dt % 2 + 1) * P],
                                h_sb[it][:, :tb],
                                start=(it == 0), stop=(it == KI - 1),
                            )
                        z_sb = zpool.tile([P, TB], F32, tag="z", name=f"z_{pfx}_{t0}_{dt}")
                        nc.vector.tensor_copy(out=z_sb[:, :tb], in_=pz[:, :tb])
                        # y stores via gpsimd SWDGE so they never delay x loads
                        # on the ACT HWDGE queue; final block on ACT (HWDGE
                        # drains faster at the tail barrier).
                        last_block = pfx == "s" and t0 + TB >= C
                        dma_eng = nc.scalar if last_block else nc.gpsimd
                        dma_eng.dma_start(out=yT[dt * P:(dt + 1) * P, t0:t0 + tb], in_=z_sb[:, :tb])
    return nc


def _ensure_ntff_hook():
    """The agent image's antenv lacks axon_hooks; synthesize it and
    register the ctypes NTFF profiling hook so trace=True works."""
    import types
    try:
        from antenv.axon_hooks import get_axon_ntff_profile_hook  # noqa: F401
        return
    except ImportError:
        pass
    mod = types.ModuleType("antenv.axon_hooks")
    mod._hook = None

    def set_axon_ntff_profile_hook(h):
        mod._hook = h

    def get_axon_ntff_profile_hook():
        return mod._hook

    mod.set_axon_ntff_profile_hook = set_axon_ntff_profile_hook
    mod.get_axon_ntff_profile_hook = get_axon_ntff_profile_hook
    sys.modules["antenv.axon_hooks"] = mod
    try:
        from trn_agent_boot.trn_boot import _ntff_profile_via_ctypes
        h = _ntff_profile_via_ctypes("/opt/axon/libaxon_pjrt.so")
        if h is not None:
            set_axon_ntff_profile_hook(h)
    except Exception as e:
        print(f"ntff hook setup failed: {e}", file=sys.stderr)


def _route(x, gate_w):
    """float64 routing: top-2 selection, softmax weights, aux loss."""
    T = x.shape[0]
    lg = x.astype(np.float64) @ gate_w.astype(np.float64).T  # [T, E]
    sel = np.argsort(-lg, axis=-1, kind="stable")[:, :TOP_K]  # [T, 2]
    top_vals = np.take_along_axis(lg, sel, axis=1)
    ex = np.exp(top_vals - top_vals.max(axis=1, keepdims=True))
    w = ex / ex.sum(axis=1, keepdims=True)  # [T, 2]

    counts = np.zeros(NUM_EXPERTS, dtype=np.int64)
    np.add.at(counts, sel.ravel(), 1)
    f = counts / (T * TOP_K)
    el = np.exp(lg - lg.max(axis=1, keepdims=True))
    Pm = (el / el.sum(axis=1, keepdims=True)).mean(axis=0)
    aux = np.float32(NUM_EXPERTS * np.sum(f * Pm))
    return sel, w.astype(np.float32), counts, aux


def kernel(hidden_states, gate_w, shared_gate, shared_up, shared_down,
           exp_gate, exp_up, exp_down):
    from concourse.bass_utils import run_bass_kernel_spmd

    B, S, Dm = hidden_states.shape
    x = np.ascontiguousarray(np.asarray(hidden_states, dtype=np.float32).reshape(-1, Dm))
    T = x.shape[0]

    sel, w, counts, aux = _route(x, np.asarray(gate_w, dtype=np.float32))

    tok_of = []
    wt_of = []
    for e in range(NUM_EXPERTS):
        mask = sel == e  # [T, 2]
        toks = np.nonzero(mask.any(axis=1))[0]
        we = np.where(mask[toks, 0], w[toks, 0], w[toks, 1])
        tok_of.append(toks)
        wt_of.append(we.astype(np.float32))

    C_E = max(128, int(-(-counts.max() // 128)) * 128)
    C_S = T // N_CORES

    key = (C_E, C_S)
    if key not in _CACHE:
        _patch_compile()
        _CACHE[key] = _build_moe_nc(C_E, C_S, TB=512)
    nc = _CACHE[key]

    xT_bf = np.ascontiguousarray(x.T).astype(BF16_NP)  # [D, T]

    in_maps = []
    for c in range(N_CORES):
        toks = tok_of[c]
        xT_e = np.zeros((D, C_E), dtype=BF16_NP)
        xT_e[:, :len(toks)] = xT_bf[:, toks]
        xT_s = np.ascontiguousarray(xT_bf[:, c * C_S:(c + 1) * C_S])
        in_maps.append({
            "xT_e": xT_e,
            "xT_s": xT_s,
            "wg_e": np.asarray(exp_gate[c], dtype=np.float32).astype(BF16_NP),
            "wu_e": np.asarray(exp_up[c], dtype=np.float32).astype(BF16_NP),
            "wd_e": np.asarray(exp_down[c], dtype=np.float32).astype(BF16_NP),
            "wg_s": np.asarray(shared_gate, dtype=np.float32).astype(BF16_NP),
            "wu_s": np.asarray(shared_up, dtype=np.float32).astype(BF16_NP),
            "wd_s": np.asarray(shared_down, dtype=np.float32).astype(BF16_NP),
        })

    trace = os.environ.get("MOE_TRACE") == "1"
    if trace:
        _ensure_ntff_hook()
    res = run_bass_kernel_spmd(nc, in_maps, core_ids=list(range(N_CORES)), trace=trace,
                               tmpdir=os.environ.get("MOE_TRACE_DIR") or None)
    kernel.last_exec_time_ns = res.exec_time_ns

    out = np.empty((T, Dm), dtype=np.float32)
    for c in range(N_CORES):
        out[c * C_S:(c + 1) * C_S] = res.results[c]["yT_s"].T
    for e in range(NUM_EXPERTS):
        toks = tok_of[e]
        y = res.results[e]["yT_e"][:, :len(toks)].T  # [n_e, D]
        out[toks] += wt_of[e][:, None] * y

    return out.reshape(B, S, Dm), np.float32(aux)


kernel.last_exec_time_ns = None


# revision 7
# speedup vs baseline: 1.1883x; 1.0023x over previous
"""Trainium2 Bass kernel for the MoE routing problem.

Strategy (expert-parallel, host dispatch/combine):
  - Router runs on host in float64 (top-2 selection, softmax weights,
    aux load-balancing loss) — negligible FLOPs.
  - Core e computes its expert's swiglu over the tokens routed to it
    (capacity-padded so all 8 cores run one SPMD program), plus a 1/8
    token-slice of the shared expert.
  - Activations/weights are cast to bf16 on host; matmuls accumulate in
    fp32 PSUM; outputs return fp32; combine (routing-weight scatter-add)
    happens on host in fp32.

Device layout: everything transposed ([D, T]) so the matmul contraction
dim is always the SBUF partition dim and no on-device transposes are
needed (gate/up/down weight matrices are all naturally contraction-major).
"""

import os
import sys

import numpy as np

for _p in ("/opt/trn_rl_repo",):
    if _p not in sys.path and os.path.isdir(_p):
        sys.path.insert(0, _p)

import ml_dtypes

import concourse.bass as bass
import concourse.mybir as mybir
import concourse.tile as tile

BF16_NP = ml_dtypes.bfloat16

D = 1024
I = 2048
P = 128
KD = D // P  # 8
KI = I // P  # 16
NUM_EXPERTS = 8
TOP_K = 2
N_CORES = 8
BF16 = mybir.dt.bfloat16
F32 = mybir.dt.float32
AF = mybir.ActivationFunctionType

_CACHE = {}


def _split_multiwait_bir(bir_bytes):
    """This walrus build accepts only ONE sync-wait per instruction.
    Split any instruction with N>1 waits into N-1 preceding single-wait
    EventSemaphore instructions on the same engine (level-triggered
    sem-ge waits commute, so this is semantically identical)."""
    import json as _json

    j = _json.loads(bir_bytes)
    ctr = 0
    for fn in j["functions"]:
        for bb in fn["blocks"]:
            out = []
            for ins in bb["instructions"]:
                si = ins.get("sync_info")
                if si:
                    waits = si.get("on_wait") or []
                    if len(waits) > 1:
                        for w in waits[:-1]:
                            ctr += 1
                            out.append({
                                "debug": ins.get("debug", 0),
                                "engine": ins["engine"],
                                "ins": [],
                                "outs": [],
                                "name": f"{ins['name']}_sw{ctr}",
                                "opcode": "EventSemaphore",
                                "sync_info": {"on_update": [], "on_wait": [w]},
                            })
                        si["on_wait"] = [waits[-1]]
                out.append(ins)
            bb["instructions"] = out
    return _json.dumps(j).encode()


def _patch_compile():
    """Install the multi-wait splitter in front of walrus compilation."""
    import concourse.bass2jax as b2j

    if getattr(b2j, "_moe_multiwait_patched", False):
        return
    _orig = b2j.compile_bir_kernel

    def _cbk(bir_str, *a, **k):
        if isinstance(bir_str, str):
            bir_str = bir_str.encode()
        return _orig(_split_multiwait_bir(bir_str), *a, **k)

    b2j.compile_bir_kernel = _cbk
    b2j._moe_multiwait_patched = True


def _build_moe_nc(C_E: int, C_S: int, TB: int = 512, fused_silu: bool = True):
    assert C_E % P == 0 and C_S % P == 0
    nc = bass.Bass()

    xT_e = nc.declare_dram_parameter("xT_e", [D, C_E], BF16, isOutput=False)
    xT_s = nc.declare_dram_parameter("xT_s", [D, C_S], BF16, isOutput=False)
    params = {}
    for pfx in ("e", "s"):
        params[f"wg_{pfx}"] = nc.declare_dram_parameter(f"wg_{pfx}", [D, I], BF16, isOutput=False)
        params[f"wu_{pfx}"] = nc.declare_dram_parameter(f"wu_{pfx}", [D, I], BF16, isOutput=False)
        params[f"wd_{pfx}"] = nc.declare_dram_parameter(f"wd_{pfx}", [I, D], BF16, isOutput=False)
    yT_e = nc.declare_dram_parameter("yT_e", [D, C_E], F32, isOutput=True)
    yT_s = nc.declare_dram_parameter("yT_s", [D, C_S], F32, isOutput=True)

    with tile.TileContext(nc) as tc:
        with (
            tc.tile_pool(name="w", bufs=1) as wpool,
            tc.tile_pool(name="x", bufs=2) as xpool,
            tc.tile_pool(name="h", bufs=2) as hpool,
            tc.tile_pool(name="a", bufs=4) as apool,
            tc.tile_pool(name="zs", bufs=4) as zpool,
            tc.tile_pool(name="pgu", bufs=3, space="PSUM") as pgu,
            tc.tile_pool(name="pz", bufs=2, space="PSUM") as pzp,
        ):
            for pfx, xT, yT, C in (("e", xT_e, yT_e, C_E), ("s", xT_s, yT_s, C_S)):
                # Weights tiled [128 rows] x [quarter of the consumed dim] and
                # DMA'd in consumption order, so the PE is never weight-starved
                # at phase start and slots free quarter-by-quarter at the
                # phase switch. Weights ride the SP HWDGE queue alone.
                QD = KD // 4   # wd quarter: 2 dt-groups = 256 cols
                wg_sb = [[wpool.tile([P, 4 * P], BF16, tag=f"wg{k}_{q}", name=f"wg_sb_{pfx}_{k}_{q}")
                          for q in range(4)] for k in range(KD)]
                wu_sb = [[wpool.tile([P, 4 * P], BF16, tag=f"wu{k}_{q}", name=f"wu_sb_{pfx}_{k}_{q}")
                          for q in range(4)] for k in range(KD)]
                wd_sb = [[wpool.tile([P, QD * P], BF16, tag=f"wd{k}_{q}", name=f"wd_sb_{pfx}_{k}_{q}")
                          for q in range(4)] for k in range(KI)]
                # Block split: plain TB-chunks, but a <256-column tail is
                # rebalanced with the previous block (512+128 -> 320+320):
                # below N=256 the LDWEIGHTS stops hiding under the matmul
                # (~85ns/mm at N=128 vs ~53ns ideal).
                blocks = []
                rem = C
                while rem > TB:
                    blocks.append(TB)
                    rem -= TB
                if rem < 256 and blocks:
                    tot = blocks.pop() + rem
                    half = (tot // 2 // P) * P
                    blocks += [tot - half, half]
                else:
                    blocks.append(rem)
                starts = [sum(blocks[:i]) for i in range(len(blocks))]
                # First block's x loads go on the ACT queue BEFORE any
                # weight DMA lands there — the PE's very first matmuls
                # need x(b0) + wg q0 and nothing else.
                tb0 = blocks[0]
                x0_sb = [xpool.tile([P, TB], BF16, tag=f"x{k}", name=f"x_{pfx}_0_{k}") for k in range(KD)]
                for k in range(KD):
                    nc.scalar.dma_start(out=x0_sb[k][:, :tb0], in_=xT[k * P:(k + 1) * P, 0:tb0])
                for q in range(4):
                    for k in range(KD):
                        nc.sync.dma_start(out=wg_sb[k][q][:], in_=params[f"wg_{pfx}"][k * P:(k + 1) * P, q * 4 * P:(q + 1) * 4 * P])
                    for k in range(KD):
                        nc.sync.dma_start(out=wu_sb[k][q][:], in_=params[f"wu_{pfx}"][k * P:(k + 1) * P, q * 4 * P:(q + 1) * 4 * P])
                for q in range(4):
                    for k in range(KI):
                        nc.sync.dma_start(out=wd_sb[k][q][:], in_=params[f"wd_{pfx}"][k * P:(k + 1) * P, q * QD * P:(q + 1) * QD * P])

                for t0, tb in zip(starts, blocks):
                    # x loads on the Activation HWDGE queue (don't queue
                    # behind the 12MB weight burst)
                    if t0 == 0:
                        x_sb = x0_sb
                    else:
                        x_sb = [xpool.tile([P, TB], BF16, tag=f"x{k}", name=f"x_{pfx}_{t0}_{k}") for k in range(KD)]
                        for k in range(KD):
                            nc.scalar.dma_start(out=x_sb[k][:, :tb], in_=xT[k * P:(k + 1) * P, t0:t0 + tb])
                    h_sb = [hpool.tile([P, TB], BF16, tag=f"h{it}", name=f"h_{pfx}_{t0}_{it}") for it in range(KI)]
                    for it in range(KI):
                        pg = pgu.tile([P, TB], F32, tag="pg", name=f"pg_{pfx}_{t0}_{it}")
                        pu = pgu.tile([P, TB], F32, tag="pu", name=f"pu_{pfx}_{t0}_{it}")
                        for k in range(KD):
                            nc.tensor.matmul(
                                pg[:, :tb],
                                wg_sb[k][it // 4][:, (it % 4) * P:(it % 4 + 1) * P],
                                x_sb[k][:, :tb],
                                start=(k == 0), stop=(k == KD - 1),
                            )
                        for k in range(KD):
                            nc.tensor.matmul(
                                pu[:, :tb],
                                wu_sb[k][it // 4][:, (it % 4) * P:(it % 4 + 1) * P],
                                x_sb[k][:, :tb],
                                start=(k == 0), stop=(k == KD - 1),
                            )
                        h_slice = h_sb[it][:, :tb]
                        s_sb = apool.tile([P, TB], F32, tag="s", name=f"s_{pfx}_{t0}_{it}")
                        if fused_silu:
                            nc.scalar.activation(s_sb[:, :tb], pg[:, :tb], AF.Silu)
                            nc.vector.tensor_mul(h_slice, pu[:, :tb], s_sb[:, :tb])
                        else:
                            nc.scalar.activation(s_sb[:, :tb], pg[:, :tb], AF.Sigmoid)
                            t_sb = apool.tile([P, TB], F32, tag="t", name=f"t_{pfx}_{t0}_{it}")
                            nc.vector.tensor_mul(t_sb[:, :tb], pg[:, :tb], s_sb[:, :tb])
                            nc.vector.tensor_mul(h_slice, pu[:, :tb], t_sb[:, :tb])
                    for dt in range(KD):
                        pz = pzp.tile([P, TB], F32, tag="pz", name=f"pz_{pfx}_{t0}_{dt}")
                        for it in range(KI):
                            nc.tensor.matmul(
                                pz[:, :tb],
                                wd_sb[it][dt // 2][:, (dt % 2) * P:(dt % 2 + 1) * P],
                                h_sb[it][:, :tb],
                                start=(it == 0), stop=(it == KI - 1),
                            )
                        z_sb = zpool.tile([P, TB], F32, tag="z", name=f"z_{pfx}_{t0}_{dt}")
                        nc.vector.tensor_copy(out=z_sb[:, :tb], in_=pz[:, :tb])
                        # y stores via gpsimd SWDGE so they never delay x loads
                        # on the ACT HWDGE queue; final block on ACT (HWDGE
                        # drains faster at the tail barrier).
                        last_block = pfx == "s" and t0 + TB >= C
                        dma_eng = nc.scalar if last_block else nc.gpsimd
                        dma_eng.dma_start(out=yT[dt * P:(dt + 1) * P, t0:t0 + tb], in_=z_sb[:, :tb])
    return nc


def _ensure_ntff_hook():
    """The agent image's antenv lacks axon_hooks; synthesize it and
    register the ctypes NTFF profiling hook so trace=True works."""
    import types
    try:
        from antenv.axon_hooks import get_axon_ntff_profile_hook  # noqa: F401
        return
    except ImportError:
        pass
    mod = types.ModuleType("antenv.axon_hooks")
    mod._hook = None

    def set_axon_ntff_profile_hook(h):
        mod._hook = h

    def get_axon_ntff_profile_hook():
        return mod._hook

    mod.set_axon_ntff_profile_hook = set_axon_ntff_profile_hook
    mod.get_axon_ntff_profile_hook = get_axon_ntff_profile_hook
    sys.modules["antenv.axon_hooks"] = mod
    try:
        from trn_agent_boot.trn_boot import _ntff_profile_via_ctypes
        h = _ntff_profile_via_ctypes("/opt/axon/libaxon_pjrt.so")
        if h is not None:
            set_axon_ntff_profile_hook(h)
    except Exception as e:
        print(f"ntff hook setup failed: {e}", file=sys.stderr)


def _route(x, gate_w):
    """float64 routing: top-2 selection, softmax weights, aux loss."""
    T = x.shape[0]
    lg = x.astype(np.float64) @ gate_w.astype(np.float64).T  # [T, E]
    sel = np.argsort(-lg, axis=-1, kind="stable")[:, :TOP_K]  # [T, 2]
    top_vals = np.take_along_axis(lg, sel, axis=1)
    ex = np.exp(top_vals - top_vals.max(axis=1, keepdims=True))
    w = ex / ex.sum(axis=1, keepdims=True)  # [T, 2]

    counts = np.zeros(NUM_EXPERTS, dtype=np.int64)
    np.add.at(counts, sel.ravel(), 1)
    f = counts / (T * TOP_K)
    el = np.exp(lg - lg.max(axis=1, keepdims=True))
    Pm = (el / el.sum(axis=1, keepdims=True)).mean(axis=0)
    aux = np.float32(NUM_EXPERTS * np.sum(f * Pm))
    return sel, w.astype(np.float32), counts, aux


def kernel(hidden_states, gate_w, shared_gate, shared_up, shared_down,
           exp_gate, exp_up, exp_down):
    from concourse.bass_utils import run_bass_kernel_spmd

    B, S, Dm = hidden_states.shape
    x = np.ascontiguousarray(np.asarray(hidden_states, dtype=np.float32).reshape(-1, Dm))
    T = x.shape[0]

    sel, w, counts, aux = _route(x, np.asarray(gate_w, dtype=np.float32))

    tok_of = []
    wt_of = []
    for e in range(NUM_EXPERTS):
        mask = sel == e  # [T, 2]
        toks = np.nonzero(mask.any(axis=1))[0]
        we = np.where(mask[toks, 0], w[toks, 0], w[toks, 1])
        tok_of.append(toks)
        wt_of.append(we.astype(np.float32))

    C_E = max(128, int(-(-counts.max() // 128)) * 128)
    C_S = T // N_CORES

    key = (C_E, C_S)
    if key not in _CACHE:
        _patch_compile()
        _CACHE[key] = _build_moe_nc(C_E, C_S, TB=512)
    nc = _CACHE[key]

    xT_bf = np.ascontiguousarray(x.T).astype(BF16_NP)  # [D, T]

    in_maps = []
    for c in range(N_CORES):
        toks = tok_of[c]
        xT_e = np.zeros((D, C_E), dtype=BF16_NP)
        xT_e[:, :len(toks)] = xT_bf[:, toks]
        xT_s = np.ascontiguousarray(xT_bf[:, c * C_S:(c + 1) * C_S])
        in_maps.append({
            "xT_e": xT_e,
            "xT_s": xT_s,
            "wg_e": np.asarray(exp_gate[c], dtype=np.float32).astype(BF16_NP),
            "wu_e": np.asarray(exp_up[c], dtype=np.float32).astype(BF16_NP),
            "wd_e": np.asarray(exp_down[c], dtype=np.float32).astype(BF16_NP),
            "wg_s": np.asarray(shared_gate, dtype=np.float32).astype(BF16_NP),
            "wu_s": np.asarray(shared_up, dtype=np.float32).astype(BF16_NP),
            "wd_s": np.asarray(shared_down, dtype=np.float32).astype(BF16_NP),
        })

    trace = os.environ.get("MOE_TRACE") == "1"
    if trace:
        _ensure_ntff_hook()
    res = run_bass_kernel_spmd(nc, in_maps, core_ids=list(range(N_CORES)), trace=trace,
                               tmpdir=os.environ.get("MOE_TRACE_DIR") or None)
    kernel.last_exec_time_ns = res.exec_time_ns

    out = np.empty((T, Dm), dtype=np.float32)
    for c in range(N_CORES):
        out[c * C_S:(c + 1) * C_S] = res.results[c]["yT_s"].T
    for e in range(NUM_EXPERTS):
        toks = tok_of[e]
        y = res.results[e]["yT_e"][:, :len(toks)].T  # [n_e, D]
        out[toks] += wt_of[e][:, None] * y

    return out.reshape(B, S, Dm), np.float32(aux)


kernel.last_exec_time_ns = None
